# revision 15
# baseline (speedup 1.0000x reference)
"""MoLoRA (top-2 MoE LoRA routing) Trainium2 kernel.

Full inputs -> shard tokens across 8 NeuronCores -> Bass/Tile kernel per core
-> gather full output.

Math (per token):
  logits = silu(x @ W1 + b1) @ W2 + b2
  top-2 softmax weights (renormalized over the top-2) == softmax over top-2
  logits == {sigmoid(l1-l2), sigmoid(l2-l1)} on the top-2 experts;
  combined = sum_e w_e * (x @ A_e @ B_e) * 2.0 ; out = base + combined.

v6 strategy (per core, 2048 tokens; kernel is jointly DMA- and PE-bound at
~17 us/tile steady state):
  - x pre-transposed + pre-cast to fp16 on the HOST: streams xT
    [d-part, tok-free] tiles straight from HBM.  fp16 over bf16: same
    bytes, 3 more mantissa bits (dominant error = top-2 routing swaps).
  - base fp16 in, out fp16 (upcast on host): ~25 MB HBM traffic/core.
  - All matmuls fp16, PSUM fp32.  Logits computed directly token-major
    (hs chunks stationary).  Top-2 weights via one tiny sigmoid.
  - Engine balance per tile: PE 64 big MMs; DVE = 16 epilogue adds +
    2 max-reduces + lsc scale; GpSimd = the elementwise softmax ops +
    output stores; ACT = silu + tiny sigmoid + PSUM->SBUF copies.
  - finals(t-1) token-chunks interleaved BETWEEN the mm1/low blocks of
    tile t so the PSUM->DVE epilogue drains while the PE streams, and
    the PE never stalls on the o_ps WAR rotation (ps_out bufs=4).
  - PE warm-up (~24 dummy MMs on zeroed scratch) brings the HAM clock
    gate to 2.4 GHz during the initial DMA head; W1 loaded h-major in 2
    sub-transfers on the scalar HWDGE queue; x tiles in 4 sub-transfers
    on sync; base(t) load deferred off the critical head.
"""
import sys

for _p in ("/opt/trn_rl_repo",):
    if _p not in sys.path:
        sys.path.insert(0, _p)

import numpy as np
from contextlib import ExitStack

import concourse.bass as bass
import concourse.tile as tile
from concourse import bacc, mybir
from concourse.bass_utils import run_bass_kernel_spmd

FP = mybir.dt.float32
HF = mybir.dt.float16
HF_NP = np.float16
NEG_BIG = -1e30

N_CORES = 8
B_, S, D = 4, 4096, 2048
E, R, H = 5, 16, 256
SCALING = 32.0 / 16.0
TT = 512
TOK = (B_ * S) // N_CORES


def _build_nc(TOK=TOK, D=D, H=H, E=E, R=R, TT=TT, n_cores=N_CORES):
    from concourse.alu_op_type import AluOpType as A

    NCH = TT // 128
    KD = D // 128
    KH = H // 128
    NT = TOK // TT
    M = E * R
    EP = 8
    ND = D // 512
    XSUB = 2
    KG = KD // XSUB
    NWARM = 24

    assert TOK % TT == 0 and TT % 128 == 0 and D % 512 == 0 and H % 128 == 0

    nc = bacc.Bacc("TRN2", num_devices=n_cores, debug=False)

    xt_d = nc.dram_tensor("xt", [NT * 128, KD * TT], HF, kind="ExternalInput")
    base_d = nc.dram_tensor("base", [TOK, D], HF, kind="ExternalInput")
    a_d = nc.dram_tensor("a_all", [128, KD * M], HF, kind="ExternalInput")
    b_d = nc.dram_tensor("b_all", [M, D], HF, kind="ExternalInput")
    w1_d = nc.dram_tensor("w1", [128, KH * KD * 128], HF, kind="ExternalInput")
    chf_d = nc.dram_tensor("chf", [128, 128 + KH * EP + M], HF,
                           kind="ExternalInput")
    cfp_d = nc.dram_tensor("cfp", [128, KH + NCH * E], FP,
                           kind="ExternalInput")
    out_d = nc.dram_tensor("out", [TOK, D], HF, kind="ExternalOutput")

    with tile.TileContext(nc) as tc, ExitStack() as ctx:
        const = ctx.enter_context(tc.tile_pool(name="const", bufs=1))
        xt_pool = ctx.enter_context(tc.tile_pool(name="xt", bufs=2))
        base_pool = ctx.enter_context(tc.tile_pool(name="basep", bufs=3))
        out_pool = ctx.enter_context(tc.tile_pool(name="outp", bufs=2))
        hs_pool = ctx.enter_context(tc.tile_pool(name="hs", bufs=2))
        sm_pool = ctx.enter_context(tc.tile_pool(name="sm", bufs=2))
        lsc_pool = ctx.enter_context(tc.tile_pool(name="lsc", bufs=2))

        ps_h = ctx.enter_context(tc.tile_pool(name="ps_h", bufs=2, space="PSUM"))
        ps_low = ctx.enter_context(tc.tile_pool(name="ps_low", bufs=1, space="PSUM"))
        ps_vw = ctx.enter_context(tc.tile_pool(name="ps_vw", bufs=1, space="PSUM"))
        ps_out = ctx.enter_context(tc.tile_pool(name="ps_out", bufs=4, space="PSUM"))

        # --- PE warm-up on zeroed scratch: HAM to 2.4 GHz during DMA head
        scratch = const.tile([128, TT], HF)
        nc.vector.memset(scratch[:], 0.0)
        for i in range(NWARM):
            wm_ps = ps_out.tile([128, 512], FP, tag="o_ps", name="wm_ps")
            nc.tensor.matmul(
                wm_ps[:], scratch[:, 0:128], scratch[:], start=True, stop=True
            )

        # --- weights: W1 (h-major, 2 subs) + packed consts on the scalar
        # HWDGE queue; LoRA weights on gpsimd (needed mid-tile-0 onward).
        w1_sb = const.tile([128, KH, KD, 128], HF)
        for hh in range(KH):
            nc.scalar.dma_start(
                w1_sb[:, hh, :, :],
                w1_d.ap()[:, hh * KD * 128 : (hh + 1) * KD * 128].rearrange(
                    "p (k c) -> p k c", c=128
                ),
            )
        chf = const.tile([128, 128 + KH * EP + M], HF)
        nc.scalar.dma_start(chf[:], chf_d.ap())
        cfp = const.tile([128, KH + NCH * E], FP)
        nc.scalar.dma_start(cfp[:], cfp_d.ap())
        a_sb = const.tile([128, KD, M], HF)
        nc.gpsimd.dma_start(a_sb[:], a_d.ap().rearrange("p (k m) -> p k m", m=M))
        bb_sb = const.tile([M, D], HF)
        nc.gpsimd.dma_start(bb_sb[:], b_d.ap())

        ident = chf[:, 0:128]
        w2col = lambda h: chf[:, 128 + h * EP : 128 + (h + 1) * EP]
        e80_sb = chf[0:E, 128 + KH * EP : 128 + KH * EP + M]
        b1_sb = cfp[:, 0:KH]
        b2b_sb = cfp[:, KH : KH + NCH * E].rearrange("p (c e) -> p c e", e=E)

        def emit_x_load(t):
            xt_sb = xt_pool.tile([128, KD, TT], HF, name="xt_sb")
            src = xt_d.ap()[t * 128 : (t + 1) * 128, :].rearrange(
                "p (k s) -> p k s", s=TT
            )
            for g in range(XSUB):
                nc.sync.dma_start(
                    xt_sb[:, g * KG : (g + 1) * KG, :],
                    src[:, g * KG : (g + 1) * KG, :],
                )
            return xt_sb

        def emit_base_load(t):
            base_sb = base_pool.tile([128, NCH, D], HF, name="base_sb")
            nc.scalar.dma_start(
                base_sb[:],
                base_d.ap()[t * TT : (t + 1) * TT, :].rearrange(
                    "(c p) d -> p c d", p=128
                ),
            )
            return base_sb

        def emit_mm1_h(t, xt_sb, h_ps, h):
            for k in range(KD):
                nc.tensor.matmul(
                    h_ps[h][:],
                    w1_sb[:, h, k, :],
                    xt_sb[:, k, :],
                    start=(k == 0),
                    stop=(k == KD - 1),
                )

        def emit_silu_h(t, h_ps, hs_sb, h):
            nc.scalar.activation(
                hs_sb[:, h, :], h_ps[h][:],
                mybir.ActivationFunctionType.Silu,
                bias=b1_sb[:, h : h + 1], scale=1.0,
            )

        def emit_lg(t, hs_sb):
            # logits token-major: lg[tok, e], hs chunks stationary
            lg_ps = ps_h.tile([128, NCH, EP], FP, tag="hps", name="lg_ps")
            for c in range(NCH):
                for h in range(KH):
                    nc.tensor.matmul(
                        lg_ps[:, c, :],
                        hs_sb[:, h, c * 128 : (c + 1) * 128],
                        w2col(h),
                        start=(h == 0),
                        stop=(h == KH - 1),
                    )
            lg_sb = sm_pool.tile([128, NCH, EP], FP, name="lg_sb")
            nc.scalar.copy(lg_sb[:], lg_ps[:])
            return lg_sb

        def emit_softmax(t, lg_sb):
            # top-2 weights via masks + one sigmoid:
            #   d1 = Ls - max;  eq = (d1 == 0)        (top-1 mask)
            #   md = d1 + eq*NEG_BIG;  m2 = max(md) = l2 - l1  (<= 0)
            #   eq2 = (md == m2)                      (top-2nd mask)
            #   sw2 = sigmoid(m2) = silu(m2)/m2       (weight of 2nd expert;
            #       Silu keeps the ACT engine on ONE table -- no reloads)
            #   vn = eq + (eq2 - eq)*sw2
            # elementwise on GpSimd (Pool: arithmetic TT + immediate TS only),
            # the two max-reduces + reciprocal on DVE, silu on ACT.
            Ls = sm_pool.tile([128, NCH, E], FP, name="Ls")
            nc.gpsimd.tensor_tensor(Ls[:], lg_sb[:, :, 0:E], b2b_sb, A.add)
            mx1 = sm_pool.tile([128, NCH], FP, name="mx1")
            nc.vector.tensor_reduce(
                mx1[:], Ls[:], axis=mybir.AxisListType.X, op=A.max
            )
            d1 = sm_pool.tile([128, NCH, E], FP, name="d1")
            nc.gpsimd.tensor_tensor(
                d1[:], Ls[:], mx1[:].broadcast_to([128, NCH, E]), A.subtract
            )
            eq = sm_pool.tile([128, NCH, E], FP, name="eq")
            nc.gpsimd.tensor_scalar(
                eq[:], d1[:], 0.0, None, op0=A.is_equal
            )
            t1 = sm_pool.tile([128, NCH, E], FP, name="t1")
            nc.gpsimd.tensor_scalar(t1[:], eq[:], NEG_BIG, None, op0=A.mult)
            md = sm_pool.tile([128, NCH, E], FP, name="md")
            nc.gpsimd.tensor_tensor(md[:], d1[:], t1[:], A.add)
            m2 = sm_pool.tile([128, NCH], FP, name="m2")
            nc.vector.tensor_reduce(
                m2[:], md[:], axis=mybir.AxisListType.X, op=A.max
            )
            # sw2 = silu(m2c)/m2c with m2c = min(m2, -1e-6) to avoid 0/0
            m2c = sm_pool.tile([128, NCH], FP, name="m2c")
            nc.gpsimd.tensor_scalar(m2c[:], m2[:], -1e-6, None, op0=A.min)
            sil = sm_pool.tile([128, NCH], FP, name="sil")
            nc.scalar.activation(
                sil[:], m2c[:], mybir.ActivationFunctionType.Silu, scale=1.0
            )
            rcp = sm_pool.tile([128, NCH], FP, name="rcp")
            nc.vector.reciprocal(rcp[:], m2c[:])
            sw2 = sm_pool.tile([128, NCH], FP, name="sw2")
            nc.gpsimd.tensor_tensor(sw2[:], sil[:], rcp[:], A.mult)
            # eq2 = (md == m2)
            t3 = sm_pool.tile([128, NCH, E], FP, name="t3")
            nc.gpsimd.tensor_tensor(
                t3[:], md[:], m2[:].broadcast_to([128, NCH, E]), A.subtract
            )
            eq2 = sm_pool.tile([128, NCH, E], FP, name="eq2")
            nc.gpsimd.tensor_scalar(eq2[:], t3[:], 0.0, None, op0=A.is_equal)
            # vn = eq + (eq2 - eq) * sw2
            t4 = sm_pool.tile([128, NCH, E], FP, name="t4")
            nc.gpsimd.tensor_tensor(t4[:], eq2[:], eq[:], A.subtract)
            t5 = sm_pool.tile([128, NCH, E], FP, name="t5")
            nc.gpsimd.tensor_tensor(
                t5[:], t4[:], sw2[:].broadcast_to([128, NCH, E]), A.mult
            )
            vn = sm_pool.tile([128, NCH, E], HF, name="vn")
            nc.gpsimd.tensor_tensor(vn[:], t5[:], eq[:], A.add)
            return vn

        def emit_low(t, xt_sb):
            low_ps = ps_low.tile([M, TT], FP, name="low_ps")
            for k in range(KD):
                nc.tensor.matmul(
                    low_ps[:],
                    a_sb[:, k, :],
                    xt_sb[:, k, :],
                    start=(k == 0),
                    stop=(k == KD - 1),
                )
            return low_ps

        def emit_expand(t, low_ps, vn):
            vt_ps = ps_vw.tile([E, TT], HF, tag="vw", name="vt_ps")
            for c in range(NCH):
                nc.tensor.transpose(
                    vt_ps[:, c * 128 : (c + 1) * 128], vn[:, c, :], ident
                )
            vt_sb = sm_pool.tile([E, TT], HF, name="vt_sb")
            nc.scalar.copy(vt_sb[:], vt_ps[:])
            we_ps = ps_vw.tile([M, TT], FP, tag="vw", name="we_ps")
            nc.tensor.matmul(we_ps[:], e80_sb, vt_sb[:], start=True, stop=True)
            we_sb = lsc_pool.tile([M, TT], FP, name="we_sb")
            nc.scalar.copy(we_sb[:], we_ps[:])

            lsc_sb = lsc_pool.tile([M, TT], HF, name="lsc_sb")
            nc.vector.tensor_tensor(lsc_sb[:], low_ps[:], we_sb[:], A.mult)
            return lsc_sb

        def emit_final_chunk(t, lsc_sb, base_sb, c, split=False):
            o_sb = out_pool.tile([128, D], HF, name="o_sb")
            for db in range(ND):
                o_ps = ps_out.tile([128, 512], FP, tag="o_ps", name="o_ps")
                nc.tensor.matmul(
                    o_ps[:],
                    lsc_sb[:, c * 128 : (c + 1) * 128],
                    bb_sb[:, db * 512 : (db + 1) * 512],
                    start=True, stop=True,
                )
                if split:
                    # tail-only: ACT copies PSUM->SBUF, GpSimd adds, so the
                    # last tile's epilogue isn't DVE-serial
                    cp_sb = sm_pool.tile([128, 512], HF, name="cp_sb")
                    nc.scalar.copy(cp_sb[:], o_ps[:])
                    nc.gpsimd.tensor_tensor(
                        o_sb[:, db * 512 : (db + 1) * 512],
                        base_sb[:, c, db * 512 : (db + 1) * 512],
                        cp_sb[:],
                        A.add,
                    )
                else:
                    nc.vector.tensor_tensor(
                        o_sb[:, db * 512 : (db + 1) * 512],
                        base_sb[:, c, db * 512 : (db + 1) * 512],
                        o_ps[:],
                        A.add,
                    )
            return o_sb

        def emit_store(t, o_sb, c):
            tok0 = t * TT + c * 128
            nc.gpsimd.dma_start(out_d.ap()[tok0 : tok0 + 128, :], o_sb[:])

        # pipeline: finals(t-1) chunks interleaved between tile-t PE blocks
        xt_cur = emit_x_load(0)
        pending = None
        for t in range(NT):
            xt_next = emit_x_load(t + 1) if t + 1 < NT else None
            osbs = []
            if pending is not None:
                osbs.append(emit_final_chunk(*pending, 0))
            h_ps = [
                ps_h.tile([128, TT], FP, tag="hps", name=f"h_ps{h}")
                for h in range(KH)
            ]
            hs_sb = hs_pool.tile([128, KH, TT], HF, name="hs_sb")
            emit_mm1_h(t, xt_cur, h_ps, 0)
            emit_silu_h(t, h_ps, hs_sb, 0)
            if pending is not None:
                osbs.append(emit_final_chunk(*pending, 1))
            emit_mm1_h(t, xt_cur, h_ps, 1)
            emit_silu_h(t, h_ps, hs_sb, 1)
            if pending is not None:
                osbs.append(emit_final_chunk(*pending, 2))
            lg_sb = emit_lg(t, hs_sb)
            vn = emit_softmax(t, lg_sb)
            low_ps = emit_low(t, xt_cur)
            if pending is not None:
                osbs.append(emit_final_chunk(*pending, 3))
            lsc_sb = emit_expand(t, low_ps, vn)
            base_sb = emit_base_load(t)
            if pending is not None:
                for c, o_sb in enumerate(osbs):
                    emit_store(pending[0], o_sb, c)
            pending = (t, lsc_sb, base_sb)
            xt_cur = xt_next
        # tail: last tile's finals; chunk 3 via the ACT+GpSimd path so the
        # epilogue isn't a single DVE serial chain
        for c in range(NCH):
            o_sb = emit_final_chunk(*pending, c, split=(c == 3))
            emit_store(pending[0], o_sb, c)

    nc.compile()
    return nc


def _host_prep(x, base_output, A, B, W1, b1, W2, b2, n_cores=N_CORES, TT=TT,
               scaling=SCALING):
    Bb, S_, Dd = x.shape
    E_, _, R_ = A.shape
    N = Bb * S_
    TOKc = N // n_cores
    NCH = TT // 128
    KD = Dd // 128
    KH = H // 128
    NT = TOKc // TT
    M = E_ * R_
    EP = 8
    xf = np.ascontiguousarray(x.reshape(N, Dd), dtype=np.float32)
    bf = base_output.reshape(N, Dd)
    a_all = A.transpose(1, 0, 2).reshape(Dd, M)
    a_all = np.ascontiguousarray(
        a_all.reshape(KD, 128, M).transpose(1, 0, 2).reshape(128, -1),
        HF_NP)
    b_all = np.ascontiguousarray((B.reshape(M, Dd) * scaling), HF_NP)

    # w1 h-major: w1[p, h, k, c] = W1[k*128+p, h*128+c]
    w1h = (np.asarray(W1, np.float32).reshape(KD, 128, KH, 128)
           .transpose(1, 2, 0, 3).reshape(128, KH * KD * 128))

    # packed fp16 consts: [ident(128) | w2 (KH*EP) | e80 (M)]
    chf = np.zeros((128, 128 + KH * EP + M), HF_NP)
    chf[:, 0:128] = np.eye(128, dtype=HF_NP)
    w2p = (np.pad(np.asarray(W2, np.float32), ((0, 0), (0, EP - W2.shape[1])))
           .reshape(KH, 128, EP).transpose(1, 0, 2).reshape(128, KH * EP))
    chf[:, 128:128 + KH * EP] = w2p.astype(HF_NP)
    for e in range(E_):
        chf[e, 128 + KH * EP + e * R_ : 128 + KH * EP + (e + 1) * R_] = 1.0

    # packed fp32 consts: [b1 (KH) | b2 broadcast (NCH*E)]
    cfp = np.zeros((128, KH + NCH * E_), np.float32)
    cfp[:, 0:KH] = np.asarray(b1, np.float32).reshape(KH, 128).T
    cfp[:, KH:] = np.tile(np.asarray(b2, np.float32), NCH)[None, :]

    shared = {
        "a_all": a_all,
        "b_all": b_all,
        "w1": np.ascontiguousarray(w1h.astype(HF_NP)),
        "chf": chf,
        "cfp": cfp,
    }
    in_maps = []
    for i in range(n_cores):
        xc = xf[i * TOKc : (i + 1) * TOKc]  # [TOKc, D]
        # xt[t, p, k, s] = xc[t*TT+s, k*128+p]
        xt = np.ascontiguousarray(
            xc.T.reshape(KD, 128, NT, TT).transpose(2, 1, 0, 3)
            .reshape(NT * 128, KD * TT), HF_NP)
        m = dict(shared)
        m["xt"] = xt
        m["base"] = np.ascontiguousarray(bf[i * TOKc : (i + 1) * TOKc], HF_NP)
        in_maps.append(m)
    return in_maps, (N, TOKc, Dd)


_NC_CACHE = {}


def _get_nc():
    if "nc" not in _NC_CACHE:
        _NC_CACHE["nc"] = _build_nc()
    return _NC_CACHE["nc"]


def kernel(x, base_output, A, B, W1, b1, W2, b2, _trace=False):
    x = np.asarray(x)
    base_output = np.asarray(base_output)
    nc = _get_nc()
    in_maps, (N, TOKc, Dd) = _host_prep(
        np.asarray(x, np.float32), np.asarray(base_output, np.float32),
        np.asarray(A, np.float32), np.asarray(B, np.float32),
        np.asarray(W1, np.float32), np.asarray(b1, np.float32),
        np.asarray(W2, np.float32), np.asarray(b2, np.float32),
    )
    res = run_bass_kernel_spmd(
        nc, in_maps, core_ids=list(range(N_CORES)), trace=_trace
    )
    out = np.concatenate(
        [np.asarray(res.results[i]["out"], np.float32) for i in range(N_CORES)],
        axis=0)
    out = out.reshape(x.shape).astype(np.float32)
    if _trace:
        kernel._last_exec_time_ns = res.exec_time_ns
        kernel._last_results = res
    return out


# revision 16
# speedup vs baseline: 1.0412x; 1.0412x over previous
"""MoLoRA (top-2 MoE LoRA routing) Trainium2 kernel.

Full inputs -> shard tokens across 8 NeuronCores -> Bass/Tile kernel per core
-> gather full output.

Math (per token):
  logits = silu(x @ W1 + b1) @ W2 + b2
  top-2 softmax weights (renormalized over the top-2) == softmax over top-2
  logits == {sigmoid(l1-l2), sigmoid(l2-l1)} on the top-2 experts;
  combined = sum_e w_e * (x @ A_e @ B_e) * 2.0 ; out = base + combined.

v6 strategy (per core, 2048 tokens; kernel is jointly DMA- and PE-bound at
~17 us/tile steady state):
  - x pre-transposed + pre-cast to fp16 on the HOST: streams xT
    [d-part, tok-free] tiles straight from HBM.  fp16 over bf16: same
    bytes, 3 more mantissa bits (dominant error = top-2 routing swaps).
  - base fp16 in, out fp16 (upcast on host): ~25 MB HBM traffic/core.
  - All matmuls fp16, PSUM fp32.  Logits computed directly token-major
    (hs chunks stationary).  Top-2 weights via one tiny sigmoid.
  - Engine balance per tile: PE 64 big MMs; DVE = 16 epilogue adds +
    2 max-reduces + lsc scale; GpSimd = the elementwise softmax ops +
    output stores; ACT = silu + tiny sigmoid + PSUM->SBUF copies.
  - finals(t-1) token-chunks interleaved BETWEEN the mm1/low blocks of
    tile t so the PSUM->DVE epilogue drains while the PE streams, and
    the PE never stalls on the o_ps WAR rotation (ps_out bufs=4).
  - PE warm-up (~24 dummy MMs on zeroed scratch) brings the HAM clock
    gate to 2.4 GHz during the initial DMA head; W1 loaded h-major in 2
    sub-transfers on the scalar HWDGE queue; x tiles in 4 sub-transfers
    on sync; base(t) load deferred off the critical head.
"""
import sys

for _p in ("/opt/trn_rl_repo",):
    if _p not in sys.path:
        sys.path.insert(0, _p)

import numpy as np
from contextlib import ExitStack

import concourse.bass as bass
import concourse.tile as tile
from concourse import bacc, mybir
from concourse.bass_utils import run_bass_kernel_spmd

FP = mybir.dt.float32
HF = mybir.dt.float16
HF_NP = np.float16
NEG_BIG = -1e30

N_CORES = 8
B_, S, D = 4, 4096, 2048
E, R, H = 5, 16, 256
SCALING = 32.0 / 16.0
TT = 512
TOK = (B_ * S) // N_CORES


def _build_nc(TOK=TOK, D=D, H=H, E=E, R=R, TT=TT, n_cores=N_CORES):
    from concourse.alu_op_type import AluOpType as A

    NCH = TT // 128
    KD = D // 128
    KH = H // 128
    NT = TOK // TT
    M = E * R
    EP = 8
    ND = D // 512
    XSUB = 4
    KG = KD // XSUB
    NWARM = 24

    assert TOK % TT == 0 and TT % 128 == 0 and D % 512 == 0 and H % 128 == 0

    nc = bacc.Bacc("TRN2", num_devices=n_cores, debug=False)

    xt_d = nc.dram_tensor("xt", [NT * 128, KD * TT], HF, kind="ExternalInput")
    base_d = nc.dram_tensor("base", [TOK, D], HF, kind="ExternalInput")
    a_d = nc.dram_tensor("a_all", [128, KD * M], HF, kind="ExternalInput")
    b_d = nc.dram_tensor("b_all", [M, D], HF, kind="ExternalInput")
    w1_d = nc.dram_tensor("w1", [128, KH * KD * 128], HF, kind="ExternalInput")
    chf_d = nc.dram_tensor("chf", [128, 128 + KH * EP + M], HF,
                           kind="ExternalInput")
    cfp_d = nc.dram_tensor("cfp", [128, KH + NCH * E], FP,
                           kind="ExternalInput")
    out_d = nc.dram_tensor("out", [TOK, D], HF, kind="ExternalOutput")

    with tile.TileContext(nc) as tc, ExitStack() as ctx:
        const = ctx.enter_context(tc.tile_pool(name="const", bufs=1))
        xt_pool = ctx.enter_context(tc.tile_pool(name="xt", bufs=2))
        base_pool = ctx.enter_context(tc.tile_pool(name="basep", bufs=3))
        out_pool = ctx.enter_context(tc.tile_pool(name="outp", bufs=2))
        hs_pool = ctx.enter_context(tc.tile_pool(name="hs", bufs=2))
        sm_pool = ctx.enter_context(tc.tile_pool(name="sm", bufs=2))
        lsc_pool = ctx.enter_context(tc.tile_pool(name="lsc", bufs=2))

        ps_h = ctx.enter_context(tc.tile_pool(name="ps_h", bufs=2, space="PSUM"))
        ps_low = ctx.enter_context(tc.tile_pool(name="ps_low", bufs=1, space="PSUM"))
        ps_vw = ctx.enter_context(tc.tile_pool(name="ps_vw", bufs=1, space="PSUM"))
        ps_out = ctx.enter_context(tc.tile_pool(name="ps_out", bufs=4, space="PSUM"))

        # --- PE warm-up on zeroed scratch: HAM to 2.4 GHz during DMA head
        scratch = const.tile([128, TT], HF)
        nc.vector.memset(scratch[:], 0.0)
        for i in range(NWARM):
            wm_ps = ps_out.tile([128, 512], FP, tag="o_ps", name="wm_ps")
            nc.tensor.matmul(
                wm_ps[:], scratch[:, 0:128], scratch[:], start=True, stop=True
            )

        # --- weights: W1 (h-major, 2 subs) + packed consts on the scalar
        # HWDGE queue; LoRA weights on gpsimd (needed mid-tile-0 onward).
        w1_sb = const.tile([128, KH, KD, 128], HF)
        for hh in range(KH):
            nc.scalar.dma_start(
                w1_sb[:, hh, :, :],
                w1_d.ap()[:, hh * KD * 128 : (hh + 1) * KD * 128].rearrange(
                    "p (k c) -> p k c", c=128
                ),
            )
        chf = const.tile([128, 128 + KH * EP + M], HF)
        nc.scalar.dma_start(chf[:], chf_d.ap())
        cfp = const.tile([128, KH + NCH * E], FP)
        nc.scalar.dma_start(cfp[:], cfp_d.ap())
        a_sb = const.tile([128, KD, M], HF)
        nc.gpsimd.dma_start(a_sb[:], a_d.ap().rearrange("p (k m) -> p k m", m=M))
        bb_sb = const.tile([M, D], HF)
        nc.gpsimd.dma_start(bb_sb[:], b_d.ap())

        ident = chf[:, 0:128]
        w2col = lambda h: chf[:, 128 + h * EP : 128 + (h + 1) * EP]
        e80_sb = chf[0:E, 128 + KH * EP : 128 + KH * EP + M]
        b1_sb = cfp[:, 0:KH]
        b2b_sb = cfp[:, KH : KH + NCH * E].rearrange("p (c e) -> p c e", e=E)

        def emit_x_load(t):
            xt_sb = xt_pool.tile([128, KD, TT], HF, name="xt_sb")
            src = xt_d.ap()[t * 128 : (t + 1) * 128, :].rearrange(
                "p (k s) -> p k s", s=TT
            )
            for g in range(XSUB):
                nc.sync.dma_start(
                    xt_sb[:, g * KG : (g + 1) * KG, :],
                    src[:, g * KG : (g + 1) * KG, :],
                )
            return xt_sb

        def emit_base_load(t):
            base_sb = base_pool.tile([128, NCH, D], HF, name="base_sb")
            nc.scalar.dma_start(
                base_sb[:],
                base_d.ap()[t * TT : (t + 1) * TT, :].rearrange(
                    "(c p) d -> p c d", p=128
                ),
            )
            return base_sb

        def emit_mm1_h(t, xt_sb, h_ps, h):
            for k in range(KD):
                nc.tensor.matmul(
                    h_ps[h][:],
                    w1_sb[:, h, k, :],
                    xt_sb[:, k, :],
                    start=(k == 0),
                    stop=(k == KD - 1),
                )

        def emit_silu_h(t, h_ps, hs_sb, h):
            nc.scalar.activation(
                hs_sb[:, h, :], h_ps[h][:],
                mybir.ActivationFunctionType.Silu,
                bias=b1_sb[:, h : h + 1], scale=1.0,
            )

        def emit_lg(t, hs_sb):
            # logits token-major: lg[tok, e], hs chunks stationary
            lg_ps = ps_h.tile([128, NCH, EP], FP, tag="hps", name="lg_ps")
            for c in range(NCH):
                for h in range(KH):
                    nc.tensor.matmul(
                        lg_ps[:, c, :],
                        hs_sb[:, h, c * 128 : (c + 1) * 128],
                        w2col(h),
                        start=(h == 0),
                        stop=(h == KH - 1),
                    )
            lg_sb = sm_pool.tile([128, NCH, EP], FP, name="lg_sb")
            nc.scalar.copy(lg_sb[:], lg_ps[:])
            return lg_sb

        def emit_softmax(t, lg_sb):
            # top-2 weights via masks + one sigmoid:
            #   d1 = Ls - max;  eq = (d1 == 0)        (top-1 mask)
            #   md = d1 + eq*NEG_BIG;  m2 = max(md) = l2 - l1  (<= 0)
            #   eq2 = (md == m2)                      (top-2nd mask)
            #   sw2 = sigmoid(m2) = silu(m2)/m2       (weight of 2nd expert;
            #       Silu keeps the ACT engine on ONE table -- no reloads)
            #   vn = eq + (eq2 - eq)*sw2
            # elementwise on GpSimd (Pool: arithmetic TT + immediate TS only),
            # the two max-reduces + reciprocal on DVE, silu on ACT.
            Ls = sm_pool.tile([128, NCH, E], FP, name="Ls")
            nc.gpsimd.tensor_tensor(Ls[:], lg_sb[:, :, 0:E], b2b_sb, A.add)
            mx1 = sm_pool.tile([128, NCH], FP, name="mx1")
            nc.vector.tensor_reduce(
                mx1[:], Ls[:], axis=mybir.AxisListType.X, op=A.max
            )
            d1 = sm_pool.tile([128, NCH, E], FP, name="d1")
            nc.gpsimd.tensor_tensor(
                d1[:], Ls[:], mx1[:].broadcast_to([128, NCH, E]), A.subtract
            )
            eq = sm_pool.tile([128, NCH, E], FP, name="eq")
            nc.gpsimd.tensor_scalar(
                eq[:], d1[:], 0.0, None, op0=A.is_equal
            )
            t1 = sm_pool.tile([128, NCH, E], FP, name="t1")
            nc.gpsimd.tensor_scalar(t1[:], eq[:], NEG_BIG, None, op0=A.mult)
            md = sm_pool.tile([128, NCH, E], FP, name="md")
            nc.gpsimd.tensor_tensor(md[:], d1[:], t1[:], A.add)
            m2 = sm_pool.tile([128, NCH], FP, name="m2")
            nc.vector.tensor_reduce(
                m2[:], md[:], axis=mybir.AxisListType.X, op=A.max
            )
            # sw2 = silu(m2c)/m2c with m2c = min(m2, -1e-6) to avoid 0/0
            m2c = sm_pool.tile([128, NCH], FP, name="m2c")
            nc.gpsimd.tensor_scalar(m2c[:], m2[:], -1e-6, None, op0=A.min)
            sil = sm_pool.tile([128, NCH], FP, name="sil")
            nc.scalar.activation(
                sil[:], m2c[:], mybir.ActivationFunctionType.Silu, scale=1.0
            )
            rcp = sm_pool.tile([128, NCH], FP, name="rcp")
            nc.vector.reciprocal(rcp[:], m2c[:])
            sw2 = sm_pool.tile([128, NCH], FP, name="sw2")
            nc.gpsimd.tensor_tensor(sw2[:], sil[:], rcp[:], A.mult)
            # eq2 = (md == m2)
            t3 = sm_pool.tile([128, NCH, E], FP, name="t3")
            nc.gpsimd.tensor_tensor(
                t3[:], md[:], m2[:].broadcast_to([128, NCH, E]), A.subtract
            )
            eq2 = sm_pool.tile([128, NCH, E], FP, name="eq2")
            nc.gpsimd.tensor_scalar(eq2[:], t3[:], 0.0, None, op0=A.is_equal)
            # vn = eq + (eq2 - eq) * sw2
            t4 = sm_pool.tile([128, NCH, E], FP, name="t4")
            nc.gpsimd.tensor_tensor(t4[:], eq2[:], eq[:], A.subtract)
            t5 = sm_pool.tile([128, NCH, E], FP, name="t5")
            nc.gpsimd.tensor_tensor(
                t5[:], t4[:], sw2[:].broadcast_to([128, NCH, E]), A.mult
            )
            vn = sm_pool.tile([128, NCH, E], HF, name="vn")
            nc.gpsimd.tensor_tensor(vn[:], t5[:], eq[:], A.add)
            return vn

        def emit_low(t, xt_sb):
            low_ps = ps_low.tile([M, TT], FP, name="low_ps")
            for k in range(KD):
                nc.tensor.matmul(
                    low_ps[:],
                    a_sb[:, k, :],
                    xt_sb[:, k, :],
                    start=(k == 0),
                    stop=(k == KD - 1),
                )
            return low_ps

        def emit_expand(t, low_ps, vn):
            vt_ps = ps_vw.tile([E, TT], HF, tag="vw", name="vt_ps")
            for c in range(NCH):
                nc.tensor.transpose(
                    vt_ps[:, c * 128 : (c + 1) * 128], vn[:, c, :], ident
                )
            vt_sb = sm_pool.tile([E, TT], HF, name="vt_sb")
            nc.scalar.copy(vt_sb[:], vt_ps[:])
            we_ps = ps_vw.tile([M, TT], FP, tag="vw", name="we_ps")
            nc.tensor.matmul(we_ps[:], e80_sb, vt_sb[:], start=True, stop=True)
            we_sb = lsc_pool.tile([M, TT], FP, name="we_sb")
            nc.scalar.copy(we_sb[:], we_ps[:])

            lsc_sb = lsc_pool.tile([M, TT], HF, name="lsc_sb")
            nc.vector.tensor_tensor(lsc_sb[:], low_ps[:], we_sb[:], A.mult)
            return lsc_sb

        def emit_final_chunk(t, lsc_sb, base_sb, c):
            o_sb = out_pool.tile([128, D], HF, name="o_sb")
            for db in range(ND):
                o_ps = ps_out.tile([128, 512], FP, tag="o_ps", name="o_ps")
                nc.tensor.matmul(
                    o_ps[:],
                    lsc_sb[:, c * 128 : (c + 1) * 128],
                    bb_sb[:, db * 512 : (db + 1) * 512],
                    start=True, stop=True,
                )
                nc.vector.tensor_tensor(
                    o_sb[:, db * 512 : (db + 1) * 512],
                    base_sb[:, c, db * 512 : (db + 1) * 512],
                    o_ps[:],
                    A.add,
                )
            return o_sb

        def emit_store(t, o_sb, c):
            tok0 = t * TT + c * 128
            nc.gpsimd.dma_start(out_d.ap()[tok0 : tok0 + 128, :], o_sb[:])

        # pipeline: finals(t-1) chunks interleaved between tile-t PE blocks
        xt_cur = emit_x_load(0)
        pending = None
        for t in range(NT):
            xt_next = emit_x_load(t + 1) if t + 1 < NT else None
            osbs = []
            if pending is not None:
                osbs.append(emit_final_chunk(*pending, 0))
            h_ps = [
                ps_h.tile([128, TT], FP, tag="hps", name=f"h_ps{h}")
                for h in range(KH)
            ]
            hs_sb = hs_pool.tile([128, KH, TT], HF, name="hs_sb")
            emit_mm1_h(t, xt_cur, h_ps, 0)
            emit_silu_h(t, h_ps, hs_sb, 0)
            if pending is not None:
                osbs.append(emit_final_chunk(*pending, 1))
            emit_mm1_h(t, xt_cur, h_ps, 1)
            emit_silu_h(t, h_ps, hs_sb, 1)
            if pending is not None:
                osbs.append(emit_final_chunk(*pending, 2))
            lg_sb = emit_lg(t, hs_sb)
            vn = emit_softmax(t, lg_sb)
            low_ps = emit_low(t, xt_cur)
            lsc_sb = emit_expand(t, low_ps, vn)
            base_sb = emit_base_load(t)
            if pending is not None:
                osbs.append(emit_final_chunk(*pending, 3))
                for c, o_sb in enumerate(osbs):
                    emit_store(pending[0], o_sb, c)
            pending = (t, lsc_sb, base_sb)
            xt_cur = xt_next
        for c in range(NCH):
            o_sb = emit_final_chunk(*pending, c)
            emit_store(pending[0], o_sb, c)

    nc.compile()
    return nc


def _host_prep(x, base_output, A, B, W1, b1, W2, b2, n_cores=N_CORES, TT=TT,
               scaling=SCALING):
    Bb, S_, Dd = x.shape
    E_, _, R_ = A.shape
    N = Bb * S_
    TOKc = N // n_cores
    NCH = TT // 128
    KD = Dd // 128
    KH = H // 128
    NT = TOKc // TT
    M = E_ * R_
    EP = 8
    xf = np.ascontiguousarray(x.reshape(N, Dd), dtype=np.float32)
    bf = base_output.reshape(N, Dd)
    a_all = A.transpose(1, 0, 2).reshape(Dd, M)
    a_all = np.ascontiguousarray(
        a_all.reshape(KD, 128, M).transpose(1, 0, 2).reshape(128, -1),
        HF_NP)
    b_all = np.ascontiguousarray((B.reshape(M, Dd) * scaling), HF_NP)

    # w1 h-major: w1[p, h, k, c] = W1[k*128+p, h*128+c]
    w1h = (np.asarray(W1, np.float32).reshape(KD, 128, KH, 128)
           .transpose(1, 2, 0, 3).reshape(128, KH * KD * 128))

    # packed fp16 consts: [ident(128) | w2 (KH*EP) | e80 (M)]
    chf = np.zeros((128, 128 + KH * EP + M), HF_NP)
    chf[:, 0:128] = np.eye(128, dtype=HF_NP)
    w2p = (np.pad(np.asarray(W2, np.float32), ((0, 0), (0, EP - W2.shape[1])))
           .reshape(KH, 128, EP).transpose(1, 0, 2).reshape(128, KH * EP))
    chf[:, 128:128 + KH * EP] = w2p.astype(HF_NP)
    for e in range(E_):
        chf[e, 128 + KH * EP + e * R_ : 128 + KH * EP + (e + 1) * R_] = 1.0

    # packed fp32 consts: [b1 (KH) | b2 broadcast (NCH*E)]
    cfp = np.zeros((128, KH + NCH * E_), np.float32)
    cfp[:, 0:KH] = np.asarray(b1, np.float32).reshape(KH, 128).T
    cfp[:, KH:] = np.tile(np.asarray(b2, np.float32), NCH)[None, :]

    shared = {
        "a_all": a_all,
        "b_all": b_all,
        "w1": np.ascontiguousarray(w1h.astype(HF_NP)),
        "chf": chf,
        "cfp": cfp,
    }
    in_maps = []
    for i in range(n_cores):
        xc = xf[i * TOKc : (i + 1) * TOKc]  # [TOKc, D]
        # xt[t, p, k, s] = xc[t*TT+s, k*128+p]
        xt = np.ascontiguousarray(
            xc.T.reshape(KD, 128, NT, TT).transpose(2, 1, 0, 3)
            .reshape(NT * 128, KD * TT), HF_NP)
        m = dict(shared)
        m["xt"] = xt
        m["base"] = np.ascontiguousarray(bf[i * TOKc : (i + 1) * TOKc], HF_NP)
        in_maps.append(m)
    return in_maps, (N, TOKc, Dd)


_NC_CACHE = {}


def _get_nc():
    if "nc" not in _NC_CACHE:
        _NC_CACHE["nc"] = _build_nc()
    return _NC_CACHE["nc"]


def kernel(x, base_output, A, B, W1, b1, W2, b2, _trace=False):
    x = np.asarray(x)
    base_output = np.asarray(base_output)
    nc = _get_nc()
    in_maps, (N, TOKc, Dd) = _host_prep(
        np.asarray(x, np.float32), np.asarray(base_output, np.float32),
        np.asarray(A, np.float32), np.asarray(B, np.float32),
        np.asarray(W1, np.float32), np.asarray(b1, np.float32),
        np.asarray(W2, np.float32), np.asarray(b2, np.float32),
    )
    res = run_bass_kernel_spmd(
        nc, in_maps, core_ids=list(range(N_CORES)), trace=_trace
    )
    out = np.concatenate(
        [np.asarray(res.results[i]["out"], np.float32) for i in range(N_CORES)],
        axis=0)
    out = out.reshape(x.shape).astype(np.float32)
    if _trace:
        kernel._last_exec_time_ns = res.exec_time_ns
        kernel._last_results = res
    return out


# revision 17
# speedup vs baseline: 1.1076x; 1.0639x over previous
"""MoLoRA (top-2 MoE LoRA routing) Trainium2 kernel.

Full inputs -> shard tokens across 8 NeuronCores -> Bass/Tile kernel per core
-> gather full output.

Math (per token):
  logits = silu(x @ W1 + b1) @ W2 + b2
  top-2 softmax weights (renormalized over the top-2) == softmax over top-2
  logits == {sigmoid(l1-l2), sigmoid(l2-l1)} on the top-2 experts;
  combined = sum_e w_e * (x @ A_e @ B_e) * 2.0 ; out = base + combined.

v6 strategy (per core, 2048 tokens; kernel is jointly DMA- and PE-bound at
~17 us/tile steady state):
  - x pre-transposed + pre-cast to fp16 on the HOST: streams xT
    [d-part, tok-free] tiles straight from HBM.  fp16 over bf16: same
    bytes, 3 more mantissa bits (dominant error = top-2 routing swaps).
  - base fp16 in, out fp16 (upcast on host): ~25 MB HBM traffic/core.
  - All matmuls fp16, PSUM fp32.  Logits computed directly token-major
    (hs chunks stationary).  Top-2 weights via one tiny sigmoid.
  - Engine balance per tile: PE 64 big MMs; DVE = 16 epilogue adds +
    2 max-reduces + lsc scale; GpSimd = the elementwise softmax ops +
    output stores; ACT = silu + tiny sigmoid + PSUM->SBUF copies.
  - finals(t-1) token-chunks interleaved BETWEEN the mm1/low blocks of
    tile t so the PSUM->DVE epilogue drains while the PE streams, and
    the PE never stalls on the o_ps WAR rotation (ps_out bufs=4).
  - PE warm-up (~24 dummy MMs on zeroed scratch) brings the HAM clock
    gate to 2.4 GHz during the initial DMA head; W1 loaded h-major in 2
    sub-transfers on the scalar HWDGE queue; x tiles in 4 sub-transfers
    on sync; base(t) load deferred off the critical head.
"""
import sys

for _p in ("/opt/trn_rl_repo",):
    if _p not in sys.path:
        sys.path.insert(0, _p)

import numpy as np
from contextlib import ExitStack

import concourse.bass as bass
import concourse.tile as tile
from concourse import bacc, mybir
from concourse.bass_utils import run_bass_kernel_spmd

FP = mybir.dt.float32
HF = mybir.dt.float16
HF_NP = np.float16
NEG_BIG = -1e30

N_CORES = 8
B_, S, D = 4, 4096, 2048
E, R, H = 5, 16, 256
SCALING = 32.0 / 16.0
TT = 512
TOK = (B_ * S) // N_CORES


def _build_nc(TOK=TOK, D=D, H=H, E=E, R=R, TT=TT, n_cores=N_CORES):
    from concourse.alu_op_type import AluOpType as A

    NCH = TT // 128
    KD = D // 128
    KH = H // 128
    NT = TOK // TT
    M = E * R
    EP = 8
    ND = D // 512
    XSUB = 4
    KG = KD // XSUB
    NWARM = 24

    assert TOK % TT == 0 and TT % 128 == 0 and D % 512 == 0 and H % 128 == 0

    nc = bacc.Bacc("TRN2", num_devices=n_cores, debug=False)

    xt_d = nc.dram_tensor("xt", [NT * 128, KD * TT], HF, kind="ExternalInput")
    base_d = nc.dram_tensor("base", [TOK, D], HF, kind="ExternalInput")
    a_d = nc.dram_tensor("a_all", [128, KD * M], HF, kind="ExternalInput")
    b_d = nc.dram_tensor("b_all", [M, D], HF, kind="ExternalInput")
    w1_d = nc.dram_tensor("w1", [128, KH * KD * 128], HF, kind="ExternalInput")
    chf_d = nc.dram_tensor("chf", [128, 128 + KH * EP + M], HF,
                           kind="ExternalInput")
    cfp_d = nc.dram_tensor("cfp", [128, KH + NCH * E], FP,
                           kind="ExternalInput")
    out_d = nc.dram_tensor("out", [TOK, D], HF, kind="ExternalOutput")

    with tile.TileContext(nc) as tc, ExitStack() as ctx:
        const = ctx.enter_context(tc.tile_pool(name="const", bufs=1))
        xt_pool = ctx.enter_context(tc.tile_pool(name="xt", bufs=2))
        base_pool = ctx.enter_context(tc.tile_pool(name="basep", bufs=3))
        out_pool = ctx.enter_context(tc.tile_pool(name="outp", bufs=2))
        hs_pool = ctx.enter_context(tc.tile_pool(name="hs", bufs=2))
        sm_pool = ctx.enter_context(tc.tile_pool(name="sm", bufs=2))
        lsc_pool = ctx.enter_context(tc.tile_pool(name="lsc", bufs=2))

        ps_h = ctx.enter_context(tc.tile_pool(name="ps_h", bufs=2, space="PSUM"))
        ps_low = ctx.enter_context(tc.tile_pool(name="ps_low", bufs=1, space="PSUM"))
        ps_vw = ctx.enter_context(tc.tile_pool(name="ps_vw", bufs=1, space="PSUM"))
        ps_out = ctx.enter_context(tc.tile_pool(name="ps_out", bufs=4, space="PSUM"))

        # --- PE warm-up on zeroed scratch: HAM to 2.4 GHz during DMA head
        scratch = const.tile([128, TT], HF)
        nc.vector.memset(scratch[:], 0.0)
        for i in range(NWARM):
            wm_ps = ps_out.tile([128, 512], FP, tag="o_ps", name="wm_ps")
            nc.tensor.matmul(
                wm_ps[:], scratch[:, 0:128], scratch[:], start=True, stop=True
            )

        # --- weights: W1 (h-major, 2 subs) + packed consts on the scalar
        # HWDGE queue; LoRA weights on gpsimd (needed mid-tile-0 onward).
        w1_sb = const.tile([128, KH, KD, 128], HF)
        for hh in range(KH):
            nc.scalar.dma_start(
                w1_sb[:, hh, :, :],
                w1_d.ap()[:, hh * KD * 128 : (hh + 1) * KD * 128].rearrange(
                    "p (k c) -> p k c", c=128
                ),
            )
        chf = const.tile([128, 128 + KH * EP + M], HF)
        nc.scalar.dma_start(chf[:], chf_d.ap())
        cfp = const.tile([128, KH + NCH * E], FP)
        nc.scalar.dma_start(cfp[:], cfp_d.ap())
        a_sb = const.tile([128, KD, M], HF)
        nc.gpsimd.dma_start(a_sb[:], a_d.ap().rearrange("p (k m) -> p k m", m=M))
        bb_sb = const.tile([M, D], HF)
        nc.gpsimd.dma_start(bb_sb[:], b_d.ap())

        ident = chf[:, 0:128]
        w2col = lambda h: chf[:, 128 + h * EP : 128 + (h + 1) * EP]
        e80_sb = chf[0:E, 128 + KH * EP : 128 + KH * EP + M]
        b1_sb = cfp[:, 0:KH]
        b2b_sb = cfp[:, KH : KH + NCH * E].rearrange("p (c e) -> p c e", e=E)

        def emit_x_load(t):
            xt_sb = xt_pool.tile([128, KD, TT], HF, name="xt_sb")
            src = xt_d.ap()[t * 128 : (t + 1) * 128, :].rearrange(
                "p (k s) -> p k s", s=TT
            )
            for g in range(XSUB):
                nc.sync.dma_start(
                    xt_sb[:, g * KG : (g + 1) * KG, :],
                    src[:, g * KG : (g + 1) * KG, :],
                )
            return xt_sb

        def emit_base_load(t):
            base_sb = base_pool.tile([128, NCH, D], HF, name="base_sb")
            nc.scalar.dma_start(
                base_sb[:],
                base_d.ap()[t * TT : (t + 1) * TT, :].rearrange(
                    "(c p) d -> p c d", p=128
                ),
            )
            return base_sb

        def emit_mm1_h(t, xt_sb, h_ps, h):
            for k in range(KD):
                nc.tensor.matmul(
                    h_ps[h][:],
                    w1_sb[:, h, k, :],
                    xt_sb[:, k, :],
                    start=(k == 0),
                    stop=(k == KD - 1),
                )

        def emit_silu_h(t, h_ps, hs_sb, h):
            nc.scalar.activation(
                hs_sb[:, h, :], h_ps[h][:],
                mybir.ActivationFunctionType.Silu,
                bias=b1_sb[:, h : h + 1], scale=1.0,
            )

        def emit_lg(t, hs_sb):
            # logits token-major: lg[tok, e], hs chunks stationary
            lg_ps = ps_h.tile([128, NCH, EP], FP, tag="hps", name="lg_ps")
            for c in range(NCH):
                for h in range(KH):
                    nc.tensor.matmul(
                        lg_ps[:, c, :],
                        hs_sb[:, h, c * 128 : (c + 1) * 128],
                        w2col(h),
                        start=(h == 0),
                        stop=(h == KH - 1),
                    )
            lg_sb = sm_pool.tile([128, NCH, EP], FP, name="lg_sb")
            nc.scalar.copy(lg_sb[:], lg_ps[:])
            return lg_sb

        def emit_softmax(t, lg_sb):
            # top-2 weights via masks + one sigmoid:
            #   d1 = Ls - max;  eq = (d1 == 0)        (top-1 mask)
            #   md = d1 + eq*NEG_BIG;  m2 = max(md) = l2 - l1  (<= 0)
            #   eq2 = (md == m2)                      (top-2nd mask)
            #   sw2 = sigmoid(m2) = silu(m2)/m2       (weight of 2nd expert;
            #       Silu keeps the ACT engine on ONE table -- no reloads)
            #   vn = eq + (eq2 - eq)*sw2
            # elementwise on GpSimd (Pool: arithmetic TT + immediate TS only),
            # the two max-reduces + reciprocal on DVE, silu on ACT.
            Ls = sm_pool.tile([128, NCH, E], FP, name="Ls")
            nc.gpsimd.tensor_tensor(Ls[:], lg_sb[:, :, 0:E], b2b_sb, A.add)
            mx1 = sm_pool.tile([128, NCH], FP, name="mx1")
            nc.vector.tensor_reduce(
                mx1[:], Ls[:], axis=mybir.AxisListType.X, op=A.max
            )
            d1 = sm_pool.tile([128, NCH, E], FP, name="d1")
            nc.gpsimd.tensor_tensor(
                d1[:], Ls[:], mx1[:].broadcast_to([128, NCH, E]), A.subtract
            )
            eq = sm_pool.tile([128, NCH, E], FP, name="eq")
            nc.gpsimd.tensor_scalar(
                eq[:], d1[:], 0.0, None, op0=A.is_equal
            )
            t1 = sm_pool.tile([128, NCH, E], FP, name="t1")
            nc.gpsimd.tensor_scalar(t1[:], eq[:], NEG_BIG, None, op0=A.mult)
            md = sm_pool.tile([128, NCH, E], FP, name="md")
            nc.gpsimd.tensor_tensor(md[:], d1[:], t1[:], A.add)
            m2 = sm_pool.tile([128, NCH], FP, name="m2")
            nc.vector.tensor_reduce(
                m2[:], md[:], axis=mybir.AxisListType.X, op=A.max
            )
            # sw2 = silu(m2c)/m2c with m2c = min(m2, -1e-6) to avoid 0/0
            m2c = sm_pool.tile([128, NCH], FP, name="m2c")
            nc.gpsimd.tensor_scalar(m2c[:], m2[:], -1e-6, None, op0=A.min)
            sil = sm_pool.tile([128, NCH], FP, name="sil")
            nc.scalar.activation(
                sil[:], m2c[:], mybir.ActivationFunctionType.Silu, scale=1.0
            )
            rcp = sm_pool.tile([128, NCH], FP, name="rcp")
            nc.vector.reciprocal(rcp[:], m2c[:])
            sw2 = sm_pool.tile([128, NCH], FP, name="sw2")
            nc.gpsimd.tensor_tensor(sw2[:], sil[:], rcp[:], A.mult)
            # eq2 = (md == m2)
            t3 = sm_pool.tile([128, NCH, E], FP, name="t3")
            nc.gpsimd.tensor_tensor(
                t3[:], md[:], m2[:].broadcast_to([128, NCH, E]), A.subtract
            )
            eq2 = sm_pool.tile([128, NCH, E], FP, name="eq2")
            nc.gpsimd.tensor_scalar(eq2[:], t3[:], 0.0, None, op0=A.is_equal)
            # vn = eq + (eq2 - eq) * sw2
            t4 = sm_pool.tile([128, NCH, E], FP, name="t4")
            nc.gpsimd.tensor_tensor(t4[:], eq2[:], eq[:], A.subtract)
            t5 = sm_pool.tile([128, NCH, E], FP, name="t5")
            nc.gpsimd.tensor_tensor(
                t5[:], t4[:], sw2[:].broadcast_to([128, NCH, E]), A.mult
            )
            vn = sm_pool.tile([128, NCH, E], HF, name="vn")
            nc.gpsimd.tensor_tensor(vn[:], t5[:], eq[:], A.add)
            return vn

        def emit_low(t, xt_sb):
            low_ps = ps_low.tile([M, TT], FP, name="low_ps")
            for k in range(KD):
                nc.tensor.matmul(
                    low_ps[:],
                    a_sb[:, k, :],
                    xt_sb[:, k, :],
                    start=(k == 0),
                    stop=(k == KD - 1),
                )
            return low_ps

        def emit_expand(t, low_ps, vn):
            vt_ps = ps_vw.tile([E, TT], HF, tag="vw", name="vt_ps")
            for c in range(NCH):
                nc.tensor.transpose(
                    vt_ps[:, c * 128 : (c + 1) * 128], vn[:, c, :], ident
                )
            vt_sb = sm_pool.tile([E, TT], HF, name="vt_sb")
            nc.scalar.copy(vt_sb[:], vt_ps[:])
            we_ps = ps_vw.tile([M, TT], FP, tag="vw", name="we_ps")
            nc.tensor.matmul(we_ps[:], e80_sb, vt_sb[:], start=True, stop=True)
            we_sb = lsc_pool.tile([M, TT], FP, name="we_sb")
            nc.scalar.copy(we_sb[:], we_ps[:])

            lsc_sb = lsc_pool.tile([M, TT], HF, name="lsc_sb")
            nc.vector.tensor_tensor(lsc_sb[:], low_ps[:], we_sb[:], A.mult)
            return lsc_sb

        def emit_final_chunk(t, lsc_sb, base_sb, c):
            o_sb = out_pool.tile([128, D], HF, name="o_sb")
            for db in range(ND):
                o_ps = ps_out.tile([128, 512], FP, tag="o_ps", name="o_ps")
                nc.tensor.matmul(
                    o_ps[:],
                    lsc_sb[:, c * 128 : (c + 1) * 128],
                    bb_sb[:, db * 512 : (db + 1) * 512],
                    start=True, stop=True,
                )
                nc.vector.tensor_tensor(
                    o_sb[:, db * 512 : (db + 1) * 512],
                    base_sb[:, c, db * 512 : (db + 1) * 512],
                    o_ps[:],
                    A.add,
                )
            tok0 = t * TT + c * 128
            nc.gpsimd.dma_start(out_d.ap()[tok0 : tok0 + 128, :], o_sb[:])

        # pipeline: finals(t-1) chunks interleaved between tile-t PE blocks
        xt_cur = emit_x_load(0)
        pending = None
        for t in range(NT):
            xt_next = emit_x_load(t + 1) if t + 1 < NT else None
            if pending is not None:
                emit_final_chunk(*pending, 0)
            h_ps = [
                ps_h.tile([128, TT], FP, tag="hps", name=f"h_ps{h}")
                for h in range(KH)
            ]
            hs_sb = hs_pool.tile([128, KH, TT], HF, name="hs_sb")
            emit_mm1_h(t, xt_cur, h_ps, 0)
            emit_silu_h(t, h_ps, hs_sb, 0)
            if pending is not None:
                emit_final_chunk(*pending, 1)
            emit_mm1_h(t, xt_cur, h_ps, 1)
            emit_silu_h(t, h_ps, hs_sb, 1)
            if pending is not None:
                emit_final_chunk(*pending, 2)
            lg_sb = emit_lg(t, hs_sb)
            vn = emit_softmax(t, lg_sb)
            low_ps = emit_low(t, xt_cur)
            lsc_sb = emit_expand(t, low_ps, vn)
            base_sb = emit_base_load(t)
            if pending is not None:
                emit_final_chunk(*pending, 3)
            pending = (t, lsc_sb, base_sb)
            xt_cur = xt_next
        for c in range(NCH):
            emit_final_chunk(*pending, c)

    nc.compile()
    return nc


def _host_prep(x, base_output, A, B, W1, b1, W2, b2, n_cores=N_CORES, TT=TT,
               scaling=SCALING):
    Bb, S_, Dd = x.shape
    E_, _, R_ = A.shape
    N = Bb * S_
    TOKc = N // n_cores
    NCH = TT // 128
    KD = Dd // 128
    KH = H // 128
    NT = TOKc // TT
    M = E_ * R_
    EP = 8
    xf = np.ascontiguousarray(x.reshape(N, Dd), dtype=np.float32)
    bf = base_output.reshape(N, Dd)
    a_all = A.transpose(1, 0, 2).reshape(Dd, M)
    a_all = np.ascontiguousarray(
        a_all.reshape(KD, 128, M).transpose(1, 0, 2).reshape(128, -1),
        HF_NP)
    b_all = np.ascontiguousarray((B.reshape(M, Dd) * scaling), HF_NP)

    # w1 h-major: w1[p, h, k, c] = W1[k*128+p, h*128+c]
    w1h = (np.asarray(W1, np.float32).reshape(KD, 128, KH, 128)
           .transpose(1, 2, 0, 3).reshape(128, KH * KD * 128))

    # packed fp16 consts: [ident(128) | w2 (KH*EP) | e80 (M)]
    chf = np.zeros((128, 128 + KH * EP + M), HF_NP)
    chf[:, 0:128] = np.eye(128, dtype=HF_NP)
    w2p = (np.pad(np.asarray(W2, np.float32), ((0, 0), (0, EP - W2.shape[1])))
           .reshape(KH, 128, EP).transpose(1, 0, 2).reshape(128, KH * EP))
    chf[:, 128:128 + KH * EP] = w2p.astype(HF_NP)
    for e in range(E_):
        chf[e, 128 + KH * EP + e * R_ : 128 + KH * EP + (e + 1) * R_] = 1.0

    # packed fp32 consts: [b1 (KH) | b2 broadcast (NCH*E)]
    cfp = np.zeros((128, KH + NCH * E_), np.float32)
    cfp[:, 0:KH] = np.asarray(b1, np.float32).reshape(KH, 128).T
    cfp[:, KH:] = np.tile(np.asarray(b2, np.float32), NCH)[None, :]

    shared = {
        "a_all": a_all,
        "b_all": b_all,
        "w1": np.ascontiguousarray(w1h.astype(HF_NP)),
        "chf": chf,
        "cfp": cfp,
    }
    in_maps = []
    for i in range(n_cores):
        xc = xf[i * TOKc : (i + 1) * TOKc]  # [TOKc, D]
        # xt[t, p, k, s] = xc[t*TT+s, k*128+p]
        xt = np.ascontiguousarray(
            xc.T.reshape(KD, 128, NT, TT).transpose(2, 1, 0, 3)
            .reshape(NT * 128, KD * TT), HF_NP)
        m = dict(shared)
        m["xt"] = xt
        m["base"] = np.ascontiguousarray(bf[i * TOKc : (i + 1) * TOKc], HF_NP)
        in_maps.append(m)
    return in_maps, (N, TOKc, Dd)


_NC_CACHE = {}


def _get_nc():
    if "nc" not in _NC_CACHE:
        _NC_CACHE["nc"] = _build_nc()
    return _NC_CACHE["nc"]


def kernel(x, base_output, A, B, W1, b1, W2, b2, _trace=False):
    x = np.asarray(x)
    base_output = np.asarray(base_output)
    nc = _get_nc()
    in_maps, (N, TOKc, Dd) = _host_prep(
        np.asarray(x, np.float32), np.asarray(base_output, np.float32),
        np.asarray(A, np.float32), np.asarray(B, np.float32),
        np.asarray(W1, np.float32), np.asarray(b1, np.float32),
        np.asarray(W2, np.float32), np.asarray(b2, np.float32),
    )
    res = run_bass_kernel_spmd(
        nc, in_maps, core_ids=list(range(N_CORES)), trace=_trace
    )
    out = np.concatenate(
        [np.asarray(res.results[i]["out"], np.float32) for i in range(N_CORES)],
        axis=0)
    out = out.reshape(x.shape).astype(np.float32)
    if _trace:
        kernel._last_exec_time_ns = res.exec_time_ns
        kernel._last_results = res
    return out
